# revision 17
# baseline (speedup 1.0000x reference)
"""Trainium2 Bass kernel for DPL safe-policy head.

Computes, for x:[B,H] and three tiny heads Wg/Wp/Wa (4/4/5 logits):
    ghost = softmax(x@Wg + bg); pacman = softmax(x@Wp + bp); base = softmax(x@Wa + ba)
    unsafe[b,a] = sum_cd pacman[b,c] * T[a,c,d] * ghost[b,d]   (T fixed 0/1 tensor)
    out = base*(1-unsafe) / sum(...)

Closed form used on device (softmax normalizations cancel except ghost/pacman's,
which fold into Sp*Sg):
    E = exp(logits), Sg = sum(EG), Sp = sum(EP), SS = Sp*Sg
    u0 = sum_c EPc*EGc ; u1 = EP0*EG1+EP2*EG3 ; u2 = EP1*EG0+EP3*EG2
    t_j = EA_j * (SS - u_j)  (u3 = u4 = 0);  out_j = t_j / sum_j t_j

Sharding: pure data parallel over batch across 8 cores (2048 rows each).

Measured design points (this container, NTFF traces):
  - A single HWDGE ring sustains ~290-300 GB/s with 4 KiB/partition lines;
    adding rings does NOT raise the aggregate (per-NC ceiling), it only
    adds DMA-queue teardown time (~1-2us each) to the fixed epilogue. So
    ALL DMAs ride the sync ring: w, b, then x in grouped transfers with up
    to 12 KiB/partition lines (bigger packets, fewer boundaries), then the
    two out transfers. The sync engine executes nothing else, so nothing
    can head-of-line-block the stream (folds/exps on ACT did, in earlier
    revisions).
  - x is uploaded PRE-TRANSPOSED and PRE-CAST to fp16 as [hp, t, c, b]
    (hp = h%128 partition, t = batch tile, c = h chunk, b = batch-in-tile)
    and kept WHOLE in SBUF (8.4 MB) -- no buffer recycling semaphores.
    h on partitions means NO on-device transposes; HBM traffic is halved
    vs fp32. Accuracy vs fp32 reference: 1.46e-3 max rel err.
  - The 21 device W columns duplicate the ghost/pacman heads in pair order
    [p0,p1,p2,p3, p0,p2,p1,p3 | g0,g1,g2,g3, g1,g3,g0,g2 | a0..a4]: the
    logic layer gets all eight EP*EG products with ONE multiply, the pair
    sums [d01,d23,u1,u2] with one [4,2]-reduce, and BOTH softmax
    denominators [sp,sp,sg,sg] with one [4,4]-reduce.
  - Per tile the PE runs a bias rank-1 matmul + 16 (FWL fp16 LDWEIGHTS +
    21-col MATMUL) accumulating into one PSUM bank; ACT copies PSUM ->
    staging (fold). Tail groups sized [5,5,5,1]: the exposed chain after
    the last matmul is a single-tile 12-op DVE chain.
"""

import numpy as np

import concourse.bass as bass
import concourse.bacc as bacc
import concourse.mybir as mybir
import concourse.tile as tile
from concourse.bass_utils import run_bass_kernel_spmd

F32 = mybir.dt.float32
F16 = mybir.dt.float16
AX = mybir.AxisListType
ADD = mybir.AluOpType.add
SUB = mybir.AluOpType.subtract

MODE = "f16t"

N_CORES = 8
B_FULL, H = 16384, 2048
B = B_FULL // N_CORES  # rows per core
P = 128
NT = B // P            # batch tiles per core
NCH = H // P           # contraction chunks
J = 21                 # 8 (EP pairs) + 8 (EG pairs) + 5 action logits
QUARTERS = (6, 6, 2, 2)  # tiles per tail group; last small = short exposed chain
# x DMA grouping: small head transfers for a fast ramp, wide middle
# transfers (12 KiB/partition) for stream efficiency, small final transfer
# so the last tile lands alone and PE finishes right behind the stream
XGROUPS = (1, 1, 2, 3, 3, 3, 2, 1)


def _build_program(mode):
    nc = bacc.Bacc("TRN2", target_bir_lowering=False, debug=False,
                   num_devices=N_CORES)
    x_d = nc.dram_tensor("x", [P, NT, NCH, P], F16, kind="ExternalInput")
    w_d = nc.dram_tensor("w", [P, NCH, J], F16, kind="ExternalInput")
    b_d = nc.dram_tensor("b", [1, J], F16, kind="ExternalInput")
    y_d = nc.dram_tensor("y", [P, NT, 5], F32, kind="ExternalOutput")

    with tile.TileContext(nc) as tc:
        with (
            tc.tile_pool(name="const", bufs=1) as cpool,
            tc.tile_pool(name="acc", bufs=7, space="PSUM") as acc_pool,
            tc.tile_pool(name="tailp", bufs=2) as tpool,
        ):
            # sync ring order: w, b, x groups, out. w/b first so the first
            # matmuls are never weight-blocked.
            w_sb = cpool.tile([P, NCH, J], F16)
            nc.sync.dma_start(w_sb[:], w_d.ap())
            b_sb = cpool.tile([1, J], F16)
            nc.sync.dma_start(b_sb[:], b_d.ap())
            ones_sb = cpool.tile([1, P], F16)
            nc.vector.memset(ones_sb[:], 1.0)

            # whole x slice stays resident in SBUF (8.4 MB of ~24)
            x_sb = cpool.tile([P, NT, NCH, P], F16)
            t0 = 0
            for g in XGROUPS:
                nc.sync.dma_start(x_sb[:, t0:t0 + g], x_d.ap()[:, t0:t0 + g])
                t0 += g

            all_st = [cpool.tile([P, n, J], F32, name=f"all_st{q}")
                      for q, n in enumerate(QUARTERS)]
            q_off = [sum(QUARTERS[:q]) for q in range(len(QUARTERS))]
            out_all = cpool.tile([P, NT, 5], F32)

            def tail(q):
                n = QUARTERS[q]
                st = all_st[q][:]
                e_all = tpool.tile([P, n, J], F32, tag=f"e_all{q}")
                nc.scalar.activation(e_all[:], st,
                                     mybir.ActivationFunctionType.Exp)
                EPd = e_all[:, :, 0:8]    # p0 p2 p1 p3 p0 p1 p2 p3
                EGd = e_all[:, :, 8:16]   # g1 g3 g0 g2 g0 g1 g2 g3
                EA = e_all[:, :, 16:21]   # device action order: a1 a2 a0 a3 a4

                # all eight EP*EG products in one multiply, then pair-reduce:
                # pr = [u1, u2, d01, d23]; u0 = d01 + d23 (added in place)
                prods = tpool.tile([P, n, 8], F32, tag=f"prods{q}")
                nc.vector.tensor_mul(prods[:], EPd, EGd)
                pr = tpool.tile([P, n, 4], F32, tag=f"pr{q}")
                nc.vector.tensor_reduce(
                    pr[:], prods[:].rearrange("p n (a b) -> p n a b", b=2),
                    axis=AX.X, op=ADD)

                # both softmax sums in one reduce: [sp, sp, sg, sg]
                spg = tpool.tile([P, n, 4], F32, tag=f"spg{q}")
                nc.vector.tensor_reduce(
                    spg[:],
                    e_all[:, :, 0:16].rearrange("p n (a b) -> p n a b", b=4),
                    axis=AX.X, op=ADD)
                ss = tpool.tile([P, n], F32, tag=f"ss{q}")
                nc.vector.tensor_mul(ss[:], spg[:, :, 0], spg[:, :, 2])

                nc.vector.tensor_tensor(pr[:, :, 2], pr[:, :, 2],
                                        pr[:, :, 3], op=ADD)
                # pr[0:3] = [u1, u2, u0] lines up with device action order
                V3 = tpool.tile([P, n, 3], F32, tag=f"V3{q}")
                nc.vector.tensor_sub(V3[:],
                                     ss[:].broadcast_to([P, n, 3]),
                                     pr[:, :, 0:3])

                tj = tpool.tile([P, n, 5], F32, tag=f"tj{q}")
                nc.vector.tensor_mul(tj[:, :, 0:3], EA[:, :, 0:3], V3[:])
                nc.vector.tensor_mul(tj[:, :, 3:5], EA[:, :, 3:5],
                                     ss[:].broadcast_to([P, n, 2]))
                s5 = tpool.tile([P, n], F32, tag=f"s5{q}")
                nc.vector.tensor_reduce(s5[:], tj[:], axis=AX.X, op=ADD)
                r5 = tpool.tile([P, n], F32, tag=f"r5{q}")
                nc.vector.reciprocal(r5[:], s5[:])
                nc.vector.tensor_mul(out_all[:, q_off[q]:q_off[q] + n, :],
                                     tj[:], r5[:].broadcast_to([P, n, 5]))

            qidx = 0
            done = 0
            for t in range(NT):
                acc = acc_pool.tile([P, J], F32)
                # bias via rank-1 matmul opens the accumulation group
                nc.tensor.matmul(acc[:], ones_sb[:], b_sb[:],
                                 start=True, stop=False)
                for c in range(NCH):
                    nc.tensor.matmul(acc[:], x_sb[:, t, c, :], w_sb[:, c, :],
                                     start=False, stop=(c == NCH - 1))
                # fold on ACT: folds never queue behind the DVE tail chains
                nc.scalar.copy(all_st[qidx][:, t - done, :], acc[:])
                if t - done == QUARTERS[qidx] - 1:
                    tail(qidx)
                    done += QUARTERS[qidx]
                    qidx += 1

            # out rides the tail of the sync ring: groups 0-2 flush while the
            # last chain runs; the final group's transfer is tiny
            n012 = sum(QUARTERS[:3])
            nc.sync.dma_start(y_d.ap()[:, 0:n012, :], out_all[:, 0:n012, :])
            nc.sync.dma_start(y_d.ap()[:, n012:NT, :], out_all[:, n012:NT, :])

    nc.compile()
    return nc


_NC_CACHE = {}


def _get_program(mode=MODE):
    if mode not in _NC_CACHE:
        _NC_CACHE[mode] = _build_program(mode)
    return _NC_CACHE[mode]


def _prep_in_maps(x, Wg, bg, Wp, bp, Wa, ba, mode=MODE):
    x = np.asarray(x, dtype=np.float32)
    Wg = np.asarray(Wg, np.float32)
    Wp = np.asarray(Wp, np.float32)
    Wa = np.asarray(Wa, np.float32)
    bg = np.asarray(bg, np.float32)
    bp = np.asarray(bp, np.float32)
    ba = np.asarray(ba, np.float32)
    # duplicated pair-order columns + permuted action columns
    # (see module docstring); prods pairs: (p0g1,p2g3)(p1g0,p3g2)
    # (p0g0,p1g1)(p2g2,p3g3) -> pr = [u1, u2, d01, d23]
    PSEL = [0, 2, 1, 3, 0, 1, 2, 3]
    GSEL = [1, 3, 0, 2, 0, 1, 2, 3]
    ASEL = [1, 2, 0, 3, 4]  # device action order: up, down, stay, left, right
    W = np.concatenate([Wp[:, PSEL], Wg[:, GSEL], Wa[:, ASEL]], axis=1)
    bvec = np.concatenate([bp[PSEL], bg[GSEL], ba[ASEL]]).reshape(1, J)
    W16 = W.astype(np.float16)
    # w uploaded pre-arranged [hp, c, j] so its DMA is contiguous
    wdev = np.ascontiguousarray(
        W16.reshape(NCH, P, J).transpose(1, 0, 2))
    b16 = bvec.astype(np.float16)
    in_maps = []
    for i in range(N_CORES):
        xc = x[i * B:(i + 1) * B].astype(np.float16)
        # [t, bp, c, hp] -> [hp, t, c, bp]
        xdev = np.ascontiguousarray(
            xc.reshape(NT, P, NCH, P).transpose(3, 0, 2, 1))
        in_maps.append({
            "x": xdev,
            "w": wdev,
            "b": b16,
        })
    return in_maps


def kernel(x, Wg, bg, Wp, bp, Wa, ba):
    in_maps = _prep_in_maps(x, Wg, bg, Wp, bp, Wa, ba)
    nc = _get_program()
    res = run_bass_kernel_spmd(nc, in_maps, core_ids=list(range(N_CORES)))
    outs = []
    for i in range(N_CORES):
        y = res.results[i]["y"]  # [P, NT, 5] in device action order
        outs.append(y.transpose(1, 0, 2).reshape(B, 5)[:, [2, 0, 1, 3, 4]])
    return np.ascontiguousarray(np.concatenate(outs, axis=0))
